# revision 8
# baseline (speedup 1.0000x reference)
"""Trainium2 Bass kernel for the CubeSimulator problem (v2).

Reference: rotate (96,96,96) grids, per-voxel line-of-sight velocity u and
intensity I, Gaussian-KDE cube over 64 velocity bins, then trilinear
downsample (96,96,64) -> (32,64,64).

Exact structure reused from v1 (validated):
 - axis0 downsample (96->32) is a pure selection of rows 3k+1;
 - axis2 downsample (64->64) is the identity;
 - axis1 downsample (96->64) is a 2-tap stencil (0.75/0.25) matmul;
 - exp(L - (v-u)^2/sig^2) = exp(A) * exp(v*B + c_v) with
   A = L + ln(norm) - u^2/sig^2, B = 2u/sig^2, c_v = -v^2/sig^2.

New in v2 (tolerance-aware, rel err ~1.5e-3 vs 2e-2 budget):
 - Coarse-bin KDE: the cube is computed at NC=32 velocity centers and all
   64 reference bins are reconstructed with a ridge-regularized
   least-squares matrix R (a Gaussian with sigma=30 sampled at dv=19 is
   ~3x oversampled; aliasing ~2e-4). Halves the dominant per-bin work.
 - Wrap layout [128, 288]: per-core voxels flat=(px*96+z) are laid out
   partition=flat%128, free=flat//128, using all 128 lanes (elementwise
   engine cost scales with free size only). The z-sum for pixel p covers
   flat [96p, 96p+96), reduced on the (otherwise idle) TensorE with three
   accumulating matmuls per bin whose [128,4] selector stationaries are
   independent of the column triplet (128*3 = 96*4).
 - KDE tiles in bf16: DVE runs 2-byte tensor_tensor at 2x; exp args stay
   fp32 (ACT reads Bt fp32; scale/bias are per-bin immediates/APs).
 - Per-bin path: one ScalarE Exp (scale=vc, bias=c_v AP) + one V/P mult
   by P0 = exp(A). ScalarE is the bottleneck engine; all of prep's
   square/abs/ln/exp stay inside the natural_log_exp_and_others table.

Sharding: 32 needed i-rows split 4-per-core across 8 cores; only the
final (64, 4*64) tile is gathered per core.
"""

import math

import numpy as np

import concourse.bacc as bacc
import concourse.bass as bass
import concourse.mybir as mybir
import concourse.tile as tile
from concourse.bass_utils import run_bass_kernel_spmd

try:
    import ml_dtypes
    _BF16 = np.dtype(ml_dtypes.bfloat16)
except Exception:  # pragma: no cover
    _BF16 = None

G = 96            # up_gal grid size
NV = 64           # reference velocity bins
NC = 32           # coarse KDE bins (reconstructed to NV by matmul)
N_CORES = 8
OUT_I = 32        # selected i rows (axis-0 downsample = row selection)
ROWS_PER_CORE = OUT_I // N_CORES   # 4
PX = ROWS_PER_CORE * G             # 384 pixels per core
NZ = G                             # z depth
NFLAT = PX * NZ                    # 36864 voxels per core
NP128 = 128
NF = NFLAT // NP128                # 288 free columns
NT = NF // 3                       # 96 column triplets (4 pixels each)
OUT_J = 64
RIDGE_LAM = 1e-4

F32 = mybir.dt.float32
BF16 = mybir.dt.bfloat16
AF = mybir.ActivationFunctionType
OP = mybir.AluOpType

LAST_EXEC_NS = None

# tuning knobs
KDE_POOL_MULTS = 6   # of the NC bf16 P0-mults, how many go to GpSimd
PREP_FUSE_STT = False # use scalar_tensor_tensor fusion in prep where legal


def _build_program(ci, si, cr, sr, sig2, lnnorm, vel, safe_affine=None):
    vel = np.asarray(vel, np.float64).reshape(-1)
    vc = np.linspace(float(vel.min()), float(vel.max()), NC)
    usc = -200.0 * si
    if safe_affine is None:
        umax2 = (200.0 * si) ** 2
        safe_affine = not (umax2 / sig2 <= 80.0)

    nc = bacc.Bacc("TRN2")

    xs = nc.dram_tensor("xs", [NP128, NF], F32, kind="ExternalInput")
    ys = nc.dram_tensor("ys", [NP128, NF], F32, kind="ExternalInput")
    zs = nc.dram_tensor("zs", [NP128, NF], F32, kind="ExternalInput")
    # per-coarse-bin exp biases c_v = -vc^2/sig^2 (replicated on partitions)
    bc = nc.dram_tensor("bc", [NP128, NC], F32, kind="ExternalInput")
    # z-reduce selector stationaries S_c[k, m] = 1{96m <= 128c+k < 96m+96}
    sc = nc.dram_tensor("sc", [NP128, 12], BF16, kind="ExternalInput")
    # reconstruction moving matrix W[(b,m), (m',v)] = delta_{m,m'} R[b, v]
    wr = nc.dram_tensor("wr", [NP128, 4 * NV], BF16, kind="ExternalInput")
    # j-downsample stationaries, zero-padded to 96 partitions:
    # sm[:, (i*4+m)*64 + jj] = wj[4s+m, jj] on rows t=24i+s, else 0
    sm = nc.dram_tensor("sm", [NT, 16 * OUT_J], BF16, kind="ExternalInput")
    # identity for the PE transpose
    idm = nc.dram_tensor("idm", [NT, NT], BF16, kind="ExternalInput")
    out = nc.dram_tensor("out", [OUT_J, ROWS_PER_CORE * NV], F32,
                         kind="ExternalOutput")

    with tile.TileContext(nc) as tc:
        with (
            tc.tile_pool(name="io", bufs=1) as io,
            tc.tile_pool(name="prep", bufs=1) as prep,
            tc.tile_pool(name="kde", bufs=2) as kde,
            tc.tile_pool(name="psum", bufs=1, space="PSUM") as psum,
        ):
            # Preload the one activation table that covers every func
            # used (ln/exp/abs): avoids two mid-kernel table swaps (~1.3us
            # each) that the auto-inserter's minimal-set choice would cause.
            from concourse.hw_specs import get_activation_tables
            tabs = get_activation_tables(nc.m.arch)
            want = {AF.Ln, AF.Exp, AF.Abs}
            set_id = None
            for idx, (tname, funcs) in enumerate(tabs.items()):
                if want.issubset(funcs):
                    set_id = idx
                    break
            if set_id is not None:
                ld = mybir.InstLoadActFuncSet(
                    name=nc.scalar.bass.get_next_instruction_name(),
                    act_func_set_id=set_id, ins=[], outs=[])
                nc.scalar.add_instruction(ld)

            xt = io.tile([NP128, NF], F32, tag="xt")
            yt = io.tile([NP128, NF], F32, tag="yt")
            zt = io.tile([NP128, NF], F32, tag="zt")
            nc.sync.dma_start(out=zt[:], in_=zs[:])
            nc.sync.dma_start(out=xt[:], in_=xs[:])
            nc.scalar.dma_start(out=yt[:], in_=ys[:])
            bct = io.tile([NP128, NC], F32, tag="bct")
            nc.scalar.dma_start(out=bct[:], in_=bc[:])
            wrt = io.tile([NP128, 4 * NV], BF16, tag="wrt")
            nc.scalar.dma_start(out=wrt[:], in_=wr[:])
            sct = io.tile([NP128, 12], BF16, tag="sct")
            nc.sync.dma_start(out=sct[:], in_=sc[:])
            smt = io.tile([NT, 16 * OUT_J], BF16, tag="smt")
            nc.sync.dma_start(out=smt[:], in_=sm[:])
            idt = io.tile([NT, NT], BF16, tag="idt")
            nc.sync.dma_start(out=idt[:], in_=idm[:])

            def vtile(name):
                return prep.tile([NP128, NF], F32, tag=name, name=name)

            # Rotated coordinates. Only rx and rz are needed:
            # r^2 = x^2+y^2+z^2 - rz^2 (rotation preserves |v|), so the
            # ry leg is dropped entirely.
            rx, rz = vtile("rx"), vtile("rz")
            sqx, sqy, sqz = vtile("sqx"), vtile("sqy"), vtile("sqz")
            s2, s3 = vtile("s2"), vtile("s3")
            xc, yc, t5, zb = vtile("xc"), vtile("yc"), vtile("t5"), vtile("zb")
            rzq, q, qs = vtile("rzq"), vtile("q"), vtile("qs")
            ya = vtile("ya")
            lnq, r = vtile("lnq"), vtile("r")
            nc.vector.tensor_mul(sqz[:], zt[:], zt[:])
            nc.vector.tensor_mul(sqx[:], xt[:], xt[:])
            nc.vector.tensor_mul(sqy[:], yt[:], yt[:])
            nc.vector.tensor_add(s2[:], sqx[:], sqy[:])
            nc.vector.tensor_add(s3[:], s2[:], sqz[:])
            # rz on Pool (xc/yc feeds from DVE to shorten the Pool chain)
            nc.vector.tensor_scalar_mul(xc[:], xt[:], si * sr)
            nc.vector.tensor_scalar_mul(yc[:], yt[:], si * cr)
            nc.gpsimd.tensor_add(t5[:], xc[:], yc[:])
            nc.gpsimd.tensor_scalar_mul(zb[:], zt[:], ci)
            nc.gpsimd.tensor_add(rz[:], t5[:], zb[:])
            nc.vector.tensor_mul(rzq[:], rz[:], rz[:])
            nc.vector.tensor_sub(q[:], s3[:], rzq[:])
            nc.vector.tensor_scalar_max(qs[:], q[:], 1e-35)
            nc.scalar.activation(lnq[:], qs[:], AF.Ln)
            nc.scalar.activation(r[:], lnq[:], AF.Exp, scale=0.5)
            # rx (needed later for u0)
            nc.vector.tensor_scalar_mul(ya[:], yt[:], -sr)
            xa = vtile("xa")
            nc.vector.tensor_scalar_mul(xa[:], xt[:], cr)
            nc.vector.tensor_add(rx[:], xa[:], ya[:])

            # u0 = rx*tanh(r/2)/r = rx*(e^r-1) / (r*(e^r+1))
            er = vtile("er")
            den, rec, t1, u0 = (vtile("den"), vtile("rec"), vtile("t1"),
                                vtile("u0"))
            nc.scalar.activation(er[:], r[:], AF.Exp)
            if PREP_FUSE_STT:
                nc.vector.scalar_tensor_tensor(den[:], er[:], 1.0, r[:],
                                               OP.add, OP.mult)
                nc.vector.reciprocal(rec[:], den[:])
                nc.gpsimd.scalar_tensor_tensor(t1[:], er[:], -1.0, rx[:],
                                               OP.add, OP.mult)
                nc.vector.tensor_mul(u0[:], t1[:], rec[:])
            else:
                ed, num = vtile("ed"), vtile("num")
                nc.vector.tensor_scalar_add(ed[:], er[:], 1.0)
                nc.vector.tensor_mul(den[:], ed[:], r[:])
                nc.vector.reciprocal(rec[:], den[:])
                nc.gpsimd.tensor_scalar_add(num[:], er[:], -1.0)
                nc.vector.tensor_mul(t1[:], rx[:], num[:])
                nc.vector.tensor_mul(u0[:], t1[:], rec[:])

            # A = L + lnnorm - (u/sig)^2 ; L = -r/3 - 2|rz| ; B = 2u/sig^2
            az, rterm, Lt = vtile("az"), vtile("rterm"), vtile("Lt")
            s1, ssq, At, Bt = (vtile("s1"), vtile("ssq"), vtile("At"),
                               vtile("Bt"))
            nc.scalar.activation(az[:], rz[:], AF.Abs)
            nc.gpsimd.tensor_scalar(rterm[:], r[:], -1.0 / 3.0, lnnorm,
                                    OP.mult, OP.add)
            if PREP_FUSE_STT:
                nc.gpsimd.scalar_tensor_tensor(Lt[:], az[:], -2.0, rterm[:],
                                               OP.mult, OP.add)
            else:
                azs = vtile("azs")
                nc.gpsimd.tensor_scalar_mul(azs[:], az[:], -2.0)
                nc.gpsimd.tensor_add(Lt[:], azs[:], rterm[:])
            nc.vector.tensor_scalar_mul(Bt[:], u0[:], usc * 2.0 / sig2)
            nc.vector.tensor_scalar_mul(s1[:], u0[:], usc / math.sqrt(sig2))
            nc.vector.tensor_mul(ssq[:], s1[:], s1[:])
            nc.vector.tensor_sub(At[:], Lt[:], ssq[:])

            P0t = vtile("P0t")
            P0b = prep.tile([NP128, NF], BF16, tag="P0b", name="P0b")

            # KDE over NC coarse bins; Op[t, 4b+m] accumulates the z-sums
            # (pixel p=4t+m of bin b) via three selector matmuls per bin
            # (Ew stationary, tiny selector moving: PSUM base partition 0).
            Op = psum.tile([NT, 4 * NC], F32)
            POOL_MULT_BINS = set(range(0, 2 * KDE_POOL_MULTS, 2))
            pending = []

            def emit_tail(b, Ew):
                eng = nc.gpsimd if b in POOL_MULT_BINS else nc.vector
                if not safe_affine:
                    e1 = Ew
                    Ew = kde.tile([NP128, NT, 3], BF16, tag="Ew", bufs=6,
                                  name=f"Ew{b}")
                    eng.tensor_mul(Ew[:], e1[:], P0b[:])
                for c in range(3):
                    nc.tensor.matmul(Op[:, 4 * b:4 * b + 4],
                                     Ew[:, :, c],
                                     sct[:, 4 * c:4 * c + 4],
                                     start=(c == 0), stop=(c == 2))

            for b in range(NC):
                vv = float(vc[b])
                if not safe_affine:
                    e1 = kde.tile([NP128, NT, 3], BF16, tag="e1", bufs=6,
                                  name=f"e1{b}")
                    nc.scalar.activation(e1[:], Bt[:], AF.Exp, scale=vv,
                                         bias=bct[:, b:b + 1])
                    pending.append((b, e1))
                else:
                    cvb = float(-vc[b] * vc[b] / sig2)
                    tmp = kde.tile([NP128, NF], F32, tag="tmpa", bufs=3,
                                   name=f"tmp{b}")
                    nc.vector.tensor_scalar(tmp[:], Bt[:], vv, cvb,
                                            OP.mult, OP.add)
                    arg = kde.tile([NP128, NF], F32, tag="arga", bufs=3,
                                   name=f"arg{b}")
                    nc.vector.tensor_add(arg[:], tmp[:], At[:])
                    Ew = kde.tile([NP128, NT, 3], BF16, tag="Ew", bufs=6,
                                  name=f"Ew{b}")
                    nc.scalar.activation(Ew[:], arg[:], AF.Exp)
                    pending.append((b, Ew))
                if b == 1 and not safe_affine:
                    # P0 exp lands on ACT only now so the first KDE exps
                    # (gated only on Bt) are not head-of-line blocked on At
                    nc.scalar.activation(P0t[:], At[:], AF.Exp)
                    nc.vector.tensor_copy(P0b[:], P0t[:])
                    for item in pending:
                        emit_tail(*item)
                    pending = []
                elif b > 1 or safe_affine:
                    for item in pending:
                        emit_tail(*item)
                    pending = []

                if b == NC // 2 - 1:
                    # first-half rearrange overlaps the second half's KDE
                    Ops0 = io.tile([NT, 2 * NC], BF16, tag="Ops0")
                    nc.vector.tensor_copy(Ops0[:], Op[:, 0:2 * NC])
                    Tp = psum.tile([4 * NC, NT], BF16)
                    nc.tensor.transpose(Tp[0:2 * NC, :], Ops0[:], idt[:])
                    Os = io.tile([NP128, NT], BF16, tag="Os")
                    nc.vector.tensor_copy(Os[0:2 * NC, :], Tp[0:2 * NC, :])
            for item in pending:
                emit_tail(*item)
            pending = []

            Ops1 = io.tile([NT, 2 * NC], BF16, tag="Ops1")
            nc.vector.tensor_copy(Ops1[:], Op[:, 2 * NC:4 * NC])
            nc.tensor.transpose(Tp[2 * NC:4 * NC, :], Ops1[:], idt[:])
            nc.vector.tensor_copy(Os[2 * NC:4 * NC, :], Tp[2 * NC:4 * NC, :])

            # cube2[px=4t+m, v] = sum_b Os[(b,m), t] * R[b, v]
            out1 = psum.tile([NT, 4 * NV], F32)
            nc.tensor.matmul(out1[:], Os[:], wrt[:], start=True, stop=True)
            Os1 = io.tile([NT, 4 * NV], BF16, tag="Os1")
            nc.vector.tensor_copy(Os1[:], out1[:])

            # j-downsample: outf[jj, (i,v)] = sum_j wj[j,jj] cube2[96i+j, v]
            outf = psum.tile([OUT_J, ROWS_PER_CORE * NV], F32)
            for i in range(ROWS_PER_CORE):
                for m in range(4):
                    nc.tensor.matmul(outf[:, NV * i:NV * (i + 1)],
                                     smt[:, (i * 4 + m) * OUT_J:
                                         (i * 4 + m + 1) * OUT_J],
                                     Os1[:, NV * m:NV * (m + 1)],
                                     start=(m == 0), stop=(m == 3))
            outf_sb = io.tile([OUT_J, ROWS_PER_CORE * NV], F32, tag="outf_sb")
            H = ROWS_PER_CORE * NV // 2
            nc.vector.tensor_copy(outf_sb[:, 0:H], outf[:, 0:H])
            nc.sync.dma_start(out=out[:, 0:H], in_=outf_sb[:, 0:H])
            nc.vector.tensor_copy(outf_sb[:, H:2 * H], outf[:, H:2 * H])
            nc.scalar.dma_start(out=out[:, H:2 * H], in_=outf_sb[:, H:2 * H])

    return nc


def _recon_matrix(vel, sig2, si):
    """Ridge-regularized reconstruction R[NC, NV]: coarse Gaussian samples
    -> fine samples, fit over all reachable centers u."""
    vel = np.asarray(vel, np.float64).reshape(-1)
    vc = np.linspace(float(vel.min()), float(vel.max()), NC)
    umax = max(200.0 * abs(si), 1e-3)
    uu = np.linspace(-umax * 1.02, umax * 1.02, 4001)
    Ac = np.exp(-((vc[None, :] - uu[:, None]) ** 2) / sig2)
    Af = np.exp(-((vel[None, :] - uu[:, None]) ** 2) / sig2)
    R = np.linalg.solve(Ac.T @ Ac + RIDGE_LAM * np.eye(NC), Ac.T @ Af)
    return R.astype(np.float32)


def kernel(**inputs):
    inc = float(np.asarray(inputs["inclination"]).reshape(-1)[0])
    rot = float(np.asarray(inputs["sky_rot"]).reshape(-1)[0])
    lb = float(np.asarray(inputs["line_broadening"]).reshape(-1)[0])
    vel = np.asarray(inputs["velocity_grid"], np.float32).reshape(-1)
    X = np.asarray(inputs["Xgrid"], np.float32)
    Y = np.asarray(inputs["Ygrid"], np.float32)
    Z = np.asarray(inputs["Zgrid"], np.float32)

    ci, si = math.cos(inc), math.sin(inc)
    cr, sr = math.cos(rot), math.sin(rot)
    sig2 = float(np.float32(lb) * np.float32(lb))
    if not (sig2 > 0.0) or not math.isfinite(sig2):
        sig2 = 1e-30  # degenerate sigma: reference output is ~0/NaN anyway
    lnnorm = float(-0.5 * math.log(2.0 * math.pi * sig2))

    nc = _build_program(ci, si, cr, sr, sig2, lnnorm, vel)
    nc.finalize()

    vc = np.linspace(float(vel.min()), float(vel.max()), NC)
    bcv = np.ascontiguousarray(
        np.tile((-(vc.astype(np.float64) ** 2) / sig2).astype(np.float32),
                (NP128, 1)))

    # selector stationaries S_c
    scv = np.zeros((NP128, 12), np.float32)
    for c in range(3):
        for k in range(NP128):
            m = (128 * c + k) // 96
            if 0 <= m < 4 and 96 * m <= 128 * c + k < 96 * (m + 1):
                scv[k, 4 * c + m] = 1.0

    # reconstruction moving matrix W[(b,m), (m',v)] = delta R[b, v]
    R = _recon_matrix(vel, sig2, si)
    wrv = np.zeros((NP128, 4 * NV), np.float32)
    for b in range(NC):
        for m in range(4):
            wrv[4 * b + m, NV * m:NV * (m + 1)] = R[b]

    # j-downsample stencil and its zero-padded stationaries
    wj = np.zeros((G, OUT_J), np.float32)
    for m in range(OUT_J // 2):
        wj[3 * m, 2 * m] = 0.75
        wj[3 * m + 1, 2 * m] = 0.25
        wj[3 * m + 1, 2 * m + 1] = 0.25
        wj[3 * m + 2, 2 * m + 1] = 0.75
    smv = np.zeros((NT, 16 * OUT_J), np.float32)
    for i in range(4):
        for m in range(4):
            col = (i * 4 + m) * OUT_J
            for s in range(24):
                smv[24 * i + s, col:col + OUT_J] = wj[4 * s + m]

    as_bf16 = (lambda a: np.ascontiguousarray(a.astype(_BF16))) if _BF16 \
        else (lambda a: np.ascontiguousarray(a))

    in_maps = []
    for c in range(N_CORES):
        rows = [3 * k + 1 for k in range(ROWS_PER_CORE * c,
                                         ROWS_PER_CORE * (c + 1))]
        def shard(a):
            s = a[rows]                      # (4, 96, 96) = (i, j, z)
            flat = s.reshape(-1)             # flat = px*96 + z
            t = flat.reshape(NF, NP128).T    # [partition, free]
            return np.ascontiguousarray(t)
        in_maps.append({"xs": shard(X), "ys": shard(Y), "zs": shard(Z),
                        "bc": bcv, "sc": as_bf16(scv), "wr": as_bf16(wrv),
                        "sm": as_bf16(smv), "idm": as_bf16(np.eye(NT, dtype=np.float32))})

    res = run_bass_kernel_spmd(nc, in_maps, core_ids=list(range(N_CORES)))
    global LAST_EXEC_NS
    LAST_EXEC_NS = res.exec_time_ns

    parts = []
    for c in range(N_CORES):
        o = res.results[c]["out"]            # (64, 256) = [jj, i*64+v]
        parts.append(o.reshape(OUT_J, ROWS_PER_CORE, NV).transpose(1, 0, 2))
    return np.concatenate(parts, axis=0).astype(np.float32)  # (32, 64, 64)


# revision 10
# speedup vs baseline: 1.0320x; 1.0320x over previous
"""Trainium2 Bass kernel for the CubeSimulator problem (v2).

Reference: rotate (96,96,96) grids, per-voxel line-of-sight velocity u and
intensity I, Gaussian-KDE cube over 64 velocity bins, then trilinear
downsample (96,96,64) -> (32,64,64).

Exact structure reused from v1 (validated):
 - axis0 downsample (96->32) is a pure selection of rows 3k+1;
 - axis2 downsample (64->64) is the identity;
 - axis1 downsample (96->64) is a 2-tap stencil (0.75/0.25) matmul;
 - exp(L - (v-u)^2/sig^2) = exp(A) * exp(v*B + c_v) with
   A = L + ln(norm) - u^2/sig^2, B = 2u/sig^2, c_v = -v^2/sig^2.

New in v2 (tolerance-aware, rel err ~1.5e-3 vs 2e-2 budget):
 - Coarse-bin KDE: the cube is computed at NC=32 velocity centers and all
   64 reference bins are reconstructed with a ridge-regularized
   least-squares matrix R (a Gaussian with sigma=30 sampled at dv=19 is
   ~3x oversampled; aliasing ~2e-4). Halves the dominant per-bin work.
 - Wrap layout [128, 288]: per-core voxels flat=(px*96+z) are laid out
   partition=flat%128, free=flat//128, using all 128 lanes (elementwise
   engine cost scales with free size only). The z-sum for pixel p covers
   flat [96p, 96p+96), reduced on the (otherwise idle) TensorE with three
   accumulating matmuls per bin whose [128,4] selector stationaries are
   independent of the column triplet (128*3 = 96*4).
 - KDE tiles in bf16: DVE runs 2-byte tensor_tensor at 2x; exp args stay
   fp32 (ACT reads Bt fp32; scale/bias are per-bin immediates/APs).
 - Per-bin path: one ScalarE Exp (scale=vc, bias=c_v AP) + one V/P mult
   by P0 = exp(A). ScalarE is the bottleneck engine; all of prep's
   square/abs/ln/exp stay inside the natural_log_exp_and_others table.

Sharding: 32 needed i-rows split 4-per-core across 8 cores; only the
final (64, 4*64) tile is gathered per core.
"""

import math

import numpy as np

import concourse.bacc as bacc
import concourse.bass as bass
import concourse.mybir as mybir
import concourse.tile as tile
from concourse.bass_utils import run_bass_kernel_spmd

try:
    import ml_dtypes
    _BF16 = np.dtype(ml_dtypes.bfloat16)
except Exception:  # pragma: no cover
    _BF16 = None

G = 96            # up_gal grid size
NV = 64           # reference velocity bins
NC = 32           # coarse KDE bins (reconstructed to NV by matmul)
N_CORES = 8
OUT_I = 32        # selected i rows (axis-0 downsample = row selection)
ROWS_PER_CORE = OUT_I // N_CORES   # 4
PX = ROWS_PER_CORE * G             # 384 pixels per core
NZ = G                             # z depth
NFLAT = PX * NZ                    # 36864 voxels per core
NP128 = 128
NF = NFLAT // NP128                # 288 free columns
NT = NF // 3                       # 96 column triplets (4 pixels each)
OUT_J = 64
RIDGE_LAM = 1e-4

F32 = mybir.dt.float32
BF16 = mybir.dt.bfloat16
AF = mybir.ActivationFunctionType
OP = mybir.AluOpType

LAST_EXEC_NS = None

# tuning knobs
KDE_POOL_MULTS = 6   # of the NC bf16 P0-mults, how many go to GpSimd
PREP_FUSE_STT = False # use scalar_tensor_tensor fusion in prep where legal


def _build_program(ci, si, cr, sr, sig2, lnnorm, vel, safe_affine=None):
    vel = np.asarray(vel, np.float64).reshape(-1)
    vc = np.linspace(float(vel.min()), float(vel.max()), NC)
    usc = -200.0 * si
    if safe_affine is None:
        umax2 = (200.0 * si) ** 2
        safe_affine = not (umax2 / sig2 <= 80.0)

    nc = bacc.Bacc("TRN2")

    xs = nc.dram_tensor("xs", [NP128, NF], F32, kind="ExternalInput")
    ys = nc.dram_tensor("ys", [NP128, NF], F32, kind="ExternalInput")
    zs = nc.dram_tensor("zs", [NP128, NF], F32, kind="ExternalInput")
    # per-coarse-bin exp biases c_v = -vc^2/sig^2 (replicated on partitions)
    bc = nc.dram_tensor("bc", [NP128, NC], F32, kind="ExternalInput")
    # z-reduce selector stationaries S_c[k, m] = 1{96m <= 128c+k < 96m+96}
    sc = nc.dram_tensor("sc", [NP128, 12], BF16, kind="ExternalInput")
    # reconstruction moving matrix W[(b,m), (m',v)] = delta_{m,m'} R[b, v]
    wr = nc.dram_tensor("wr", [NP128, 4 * NV], BF16, kind="ExternalInput")
    # j-downsample stationaries, zero-padded to 96 partitions:
    # sm[:, (i*4+m)*64 + jj] = wj[4s+m, jj] on rows t=24i+s, else 0
    sm = nc.dram_tensor("sm", [NT, 16 * OUT_J], BF16, kind="ExternalInput")
    # identity for the PE transpose
    idm = nc.dram_tensor("idm", [NT, NT], BF16, kind="ExternalInput")
    out = nc.dram_tensor("out", [OUT_J, ROWS_PER_CORE * NV], F32,
                         kind="ExternalOutput")

    with tile.TileContext(nc) as tc:
        with (
            tc.tile_pool(name="io", bufs=1) as io,
            tc.tile_pool(name="prep", bufs=1) as prep,
            tc.tile_pool(name="kde", bufs=2) as kde,
            tc.tile_pool(name="psum", bufs=1, space="PSUM") as psum,
        ):
            # Preload the one activation table that covers every func
            # used (ln/exp/abs): avoids two mid-kernel table swaps (~1.3us
            # each) that the auto-inserter's minimal-set choice would cause.
            from concourse.hw_specs import get_activation_tables
            tabs = get_activation_tables(nc.m.arch)
            want = {AF.Ln, AF.Exp, AF.Abs}
            set_id = None
            for idx, (tname, funcs) in enumerate(tabs.items()):
                if want.issubset(funcs):
                    set_id = idx
                    break
            if set_id is not None:
                ld = mybir.InstLoadActFuncSet(
                    name=nc.scalar.bass.get_next_instruction_name(),
                    act_func_set_id=set_id, ins=[], outs=[])
                nc.scalar.add_instruction(ld)

            xt = io.tile([NP128, NF], F32, tag="xt")
            yt = io.tile([NP128, NF], F32, tag="yt")
            zt = io.tile([NP128, NF], F32, tag="zt")
            nc.sync.dma_start(out=zt[:], in_=zs[:])
            nc.sync.dma_start(out=xt[:], in_=xs[:])
            nc.sync.dma_start(out=yt[:], in_=ys[:])
            bct = io.tile([NP128, NC], F32, tag="bct")
            nc.sync.dma_start(out=bct[:], in_=bc[:])
            wrt = io.tile([NP128, 4 * NV], BF16, tag="wrt")
            nc.sync.dma_start(out=wrt[:], in_=wr[:])
            sct = io.tile([NP128, 12], BF16, tag="sct")
            nc.sync.dma_start(out=sct[:], in_=sc[:])
            smt = io.tile([NT, 16 * OUT_J], BF16, tag="smt")
            nc.sync.dma_start(out=smt[:], in_=sm[:])
            idt = io.tile([NT, NT], BF16, tag="idt")
            nc.sync.dma_start(out=idt[:], in_=idm[:])

            def vtile(name):
                return prep.tile([NP128, NF], F32, tag=name, name=name)

            # Rotated coordinates. Only rx and rz are needed:
            # r^2 = x^2+y^2+z^2 - rz^2 (rotation preserves |v|), so the
            # ry leg is dropped entirely.
            rx, rz = vtile("rx"), vtile("rz")
            sqx, sqy, sqz = vtile("sqx"), vtile("sqy"), vtile("sqz")
            s2, s3 = vtile("s2"), vtile("s3")
            xc, yc, t5, zb = vtile("xc"), vtile("yc"), vtile("t5"), vtile("zb")
            rzq, q, qs = vtile("rzq"), vtile("q"), vtile("qs")
            ya = vtile("ya")
            lnq, r = vtile("lnq"), vtile("r")
            nc.vector.tensor_mul(sqz[:], zt[:], zt[:])
            nc.vector.tensor_mul(sqx[:], xt[:], xt[:])
            nc.vector.tensor_mul(sqy[:], yt[:], yt[:])
            nc.vector.tensor_add(s2[:], sqx[:], sqy[:])
            nc.vector.tensor_add(s3[:], s2[:], sqz[:])
            # rz on Pool (xc/yc feeds from DVE to shorten the Pool chain)
            nc.vector.tensor_scalar_mul(xc[:], xt[:], si * sr)
            nc.vector.tensor_scalar_mul(yc[:], yt[:], si * cr)
            nc.gpsimd.tensor_add(t5[:], xc[:], yc[:])
            nc.gpsimd.tensor_scalar_mul(zb[:], zt[:], ci)
            nc.gpsimd.tensor_add(rz[:], t5[:], zb[:])
            nc.vector.tensor_mul(rzq[:], rz[:], rz[:])
            nc.vector.tensor_sub(q[:], s3[:], rzq[:])
            nc.vector.tensor_scalar_max(qs[:], q[:], 1e-35)
            nc.scalar.activation(lnq[:], qs[:], AF.Ln)
            nc.scalar.activation(r[:], lnq[:], AF.Exp, scale=0.5)
            # rx (needed later for u0)
            nc.vector.tensor_scalar_mul(ya[:], yt[:], -sr)
            xa = vtile("xa")
            nc.vector.tensor_scalar_mul(xa[:], xt[:], cr)
            nc.vector.tensor_add(rx[:], xa[:], ya[:])

            # u0 = rx*tanh(r/2)/r = rx*(e^r-1) / (r*(e^r+1))
            er = vtile("er")
            den, rec, t1, u0 = (vtile("den"), vtile("rec"), vtile("t1"),
                                vtile("u0"))
            nc.scalar.activation(er[:], r[:], AF.Exp)
            if PREP_FUSE_STT:
                nc.vector.scalar_tensor_tensor(den[:], er[:], 1.0, r[:],
                                               OP.add, OP.mult)
                nc.vector.reciprocal(rec[:], den[:])
                nc.gpsimd.scalar_tensor_tensor(t1[:], er[:], -1.0, rx[:],
                                               OP.add, OP.mult)
                nc.vector.tensor_mul(u0[:], t1[:], rec[:])
            else:
                ed, num = vtile("ed"), vtile("num")
                nc.vector.tensor_scalar_add(ed[:], er[:], 1.0)
                nc.vector.tensor_mul(den[:], ed[:], r[:])
                nc.vector.reciprocal(rec[:], den[:])
                nc.gpsimd.tensor_scalar_add(num[:], er[:], -1.0)
                nc.gpsimd.tensor_mul(t1[:], rx[:], num[:])
                nc.vector.tensor_mul(u0[:], t1[:], rec[:])

            # A = L + lnnorm - (u/sig)^2 ; L = -r/3 - 2|rz| ; B = 2u/sig^2
            az, rterm, Lt = vtile("az"), vtile("rterm"), vtile("Lt")
            s1, ssq, At = vtile("s1"), vtile("ssq"), vtile("At")
            nc.scalar.activation(az[:], rz[:], AF.Abs)
            nc.gpsimd.tensor_scalar(rterm[:], r[:], -1.0 / 3.0, lnnorm,
                                    OP.mult, OP.add)
            if PREP_FUSE_STT:
                nc.gpsimd.scalar_tensor_tensor(Lt[:], az[:], -2.0, rterm[:],
                                               OP.mult, OP.add)
            else:
                azs = vtile("azs")
                nc.gpsimd.tensor_scalar_mul(azs[:], az[:], -2.0)
                nc.gpsimd.tensor_add(Lt[:], azs[:], rterm[:])
            nc.vector.tensor_scalar_mul(s1[:], u0[:], usc / math.sqrt(sig2))
            nc.vector.tensor_mul(ssq[:], s1[:], s1[:])
            nc.vector.tensor_sub(At[:], Lt[:], ssq[:])

            P0t = vtile("P0t")
            P0b = prep.tile([NP128, NF], BF16, tag="P0b", name="P0b")

            # KDE over NC coarse bins; Op[t, 4b+m] accumulates the z-sums
            # (pixel p=4t+m of bin b) via three selector matmuls per bin
            # (Ew stationary, tiny selector moving: PSUM base partition 0).
            Op = psum.tile([NT, 4 * NC], F32)
            POOL_MULT_BINS = set(range(0, 2 * KDE_POOL_MULTS, 2))
            pending = []

            def emit_tail(b, Ew):
                eng = nc.gpsimd if b in POOL_MULT_BINS else nc.vector
                if not safe_affine:
                    e1 = Ew
                    Ew = kde.tile([NP128, NT, 3], BF16, tag="Ew", bufs=6,
                                  name=f"Ew{b}")
                    eng.tensor_mul(Ew[:], e1[:], P0b[:])
                for c in range(3):
                    nc.tensor.matmul(Op[:, 4 * b:4 * b + 4],
                                     Ew[:, :, c],
                                     sct[:, 4 * c:4 * c + 4],
                                     start=(c == 0), stop=(c == 2))

            for b in range(NC):
                vv = float(vc[b])
                if not safe_affine:
                    e1 = kde.tile([NP128, NT, 3], BF16, tag="e1", bufs=6,
                                  name=f"e1{b}")
                    nc.scalar.activation(e1[:], u0[:], AF.Exp,
                                         scale=vv * usc * 2.0 / sig2,
                                         bias=bct[:, b:b + 1])
                    pending.append((b, e1))
                else:
                    cvb = float(-vc[b] * vc[b] / sig2)
                    tmp = kde.tile([NP128, NF], F32, tag="tmpa", bufs=3,
                                   name=f"tmp{b}")
                    nc.vector.tensor_scalar(tmp[:], u0[:],
                                            vv * usc * 2.0 / sig2, cvb,
                                            OP.mult, OP.add)
                    arg = kde.tile([NP128, NF], F32, tag="arga", bufs=3,
                                   name=f"arg{b}")
                    nc.vector.tensor_add(arg[:], tmp[:], At[:])
                    Ew = kde.tile([NP128, NT, 3], BF16, tag="Ew", bufs=6,
                                  name=f"Ew{b}")
                    nc.scalar.activation(Ew[:], arg[:], AF.Exp)
                    pending.append((b, Ew))
                if b == 1 and not safe_affine:
                    # P0 exp lands on ACT only now so the first KDE exps
                    # (gated only on Bt) are not head-of-line blocked on At
                    nc.scalar.activation(P0t[:], At[:], AF.Exp)
                    nc.vector.tensor_copy(P0b[:], P0t[:])
                    for item in pending:
                        emit_tail(*item)
                    pending = []
                elif b > 1 or safe_affine:
                    for item in pending:
                        emit_tail(*item)
                    pending = []

                if b == NC // 2 - 1:
                    # first-half rearrange + W-matmul overlap the 2nd half
                    Ops0 = io.tile([NT, 2 * NC], BF16, tag="Ops0")
                    nc.vector.tensor_copy(Ops0[:], Op[:, 0:2 * NC])
                    Tp = psum.tile([4 * NC, NT], BF16)
                    nc.tensor.transpose(Tp[0:2 * NC, :], Ops0[:], idt[:])
                    Os = io.tile([NP128, NT], BF16, tag="Os")
                    nc.vector.tensor_copy(Os[0:2 * NC, :], Tp[0:2 * NC, :])
                    out1 = psum.tile([NT, 4 * NV], F32)
                    nc.tensor.matmul(out1[:], Os[0:2 * NC, :],
                                     wrt[0:2 * NC, :],
                                     start=True, stop=False)
            for item in pending:
                emit_tail(*item)
            pending = []

            Ops1 = io.tile([NT, 2 * NC], BF16, tag="Ops1")
            nc.vector.tensor_copy(Ops1[:], Op[:, 2 * NC:4 * NC])
            nc.tensor.transpose(Tp[2 * NC:4 * NC, :], Ops1[:], idt[:])
            nc.vector.tensor_copy(Os[2 * NC:4 * NC, :], Tp[2 * NC:4 * NC, :])
            nc.tensor.matmul(out1[:], Os[2 * NC:4 * NC, :],
                             wrt[2 * NC:4 * NC, :],
                             start=False, stop=True)

            Os1 = io.tile([NT, 4 * NV], BF16, tag="Os1")
            nc.vector.tensor_copy(Os1[:], out1[:])

            # j-downsample: outf[jj, (i,v)] = sum_j wj[j,jj] cube2[96i+j, v]
            outf = psum.tile([OUT_J, ROWS_PER_CORE * NV], F32)
            for i in range(ROWS_PER_CORE):
                for m in range(4):
                    nc.tensor.matmul(outf[:, NV * i:NV * (i + 1)],
                                     smt[:, (i * 4 + m) * OUT_J:
                                         (i * 4 + m + 1) * OUT_J],
                                     Os1[:, NV * m:NV * (m + 1)],
                                     start=(m == 0), stop=(m == 3))
            outf_sb = io.tile([OUT_J, ROWS_PER_CORE * NV], F32, tag="outf_sb")
            H = ROWS_PER_CORE * NV // 2
            nc.vector.tensor_copy(outf_sb[:, 0:H], outf[:, 0:H])
            nc.sync.dma_start(out=out[:, 0:H], in_=outf_sb[:, 0:H])
            nc.vector.tensor_copy(outf_sb[:, H:2 * H], outf[:, H:2 * H])
            nc.sync.dma_start(out=out[:, H:2 * H], in_=outf_sb[:, H:2 * H])

    return nc


def _recon_matrix(vel, sig2, si):
    """Ridge-regularized reconstruction R[NC, NV]: coarse Gaussian samples
    -> fine samples, fit over all reachable centers u."""
    vel = np.asarray(vel, np.float64).reshape(-1)
    vc = np.linspace(float(vel.min()), float(vel.max()), NC)
    umax = max(200.0 * abs(si), 1e-3)
    uu = np.linspace(-umax * 1.02, umax * 1.02, 4001)
    Ac = np.exp(-((vc[None, :] - uu[:, None]) ** 2) / sig2)
    Af = np.exp(-((vel[None, :] - uu[:, None]) ** 2) / sig2)
    R = np.linalg.solve(Ac.T @ Ac + RIDGE_LAM * np.eye(NC), Ac.T @ Af)
    return R.astype(np.float32)


def kernel(**inputs):
    inc = float(np.asarray(inputs["inclination"]).reshape(-1)[0])
    rot = float(np.asarray(inputs["sky_rot"]).reshape(-1)[0])
    lb = float(np.asarray(inputs["line_broadening"]).reshape(-1)[0])
    vel = np.asarray(inputs["velocity_grid"], np.float32).reshape(-1)
    X = np.asarray(inputs["Xgrid"], np.float32)
    Y = np.asarray(inputs["Ygrid"], np.float32)
    Z = np.asarray(inputs["Zgrid"], np.float32)

    ci, si = math.cos(inc), math.sin(inc)
    cr, sr = math.cos(rot), math.sin(rot)
    sig2 = float(np.float32(lb) * np.float32(lb))
    if not (sig2 > 0.0) or not math.isfinite(sig2):
        sig2 = 1e-30  # degenerate sigma: reference output is ~0/NaN anyway
    lnnorm = float(-0.5 * math.log(2.0 * math.pi * sig2))

    nc = _build_program(ci, si, cr, sr, sig2, lnnorm, vel)
    nc.finalize()

    vc = np.linspace(float(vel.min()), float(vel.max()), NC)
    bcv = np.ascontiguousarray(
        np.tile((-(vc.astype(np.float64) ** 2) / sig2).astype(np.float32),
                (NP128, 1)))

    # selector stationaries S_c
    scv = np.zeros((NP128, 12), np.float32)
    for c in range(3):
        for k in range(NP128):
            m = (128 * c + k) // 96
            if 0 <= m < 4 and 96 * m <= 128 * c + k < 96 * (m + 1):
                scv[k, 4 * c + m] = 1.0

    # reconstruction moving matrix W[(b,m), (m',v)] = delta R[b, v]
    R = _recon_matrix(vel, sig2, si)
    wrv = np.zeros((NP128, 4 * NV), np.float32)
    for b in range(NC):
        for m in range(4):
            wrv[4 * b + m, NV * m:NV * (m + 1)] = R[b]

    # j-downsample stencil and its zero-padded stationaries
    wj = np.zeros((G, OUT_J), np.float32)
    for m in range(OUT_J // 2):
        wj[3 * m, 2 * m] = 0.75
        wj[3 * m + 1, 2 * m] = 0.25
        wj[3 * m + 1, 2 * m + 1] = 0.25
        wj[3 * m + 2, 2 * m + 1] = 0.75
    smv = np.zeros((NT, 16 * OUT_J), np.float32)
    for i in range(4):
        for m in range(4):
            col = (i * 4 + m) * OUT_J
            for s in range(24):
                smv[24 * i + s, col:col + OUT_J] = wj[4 * s + m]

    as_bf16 = (lambda a: np.ascontiguousarray(a.astype(_BF16))) if _BF16 \
        else (lambda a: np.ascontiguousarray(a))

    in_maps = []
    for c in range(N_CORES):
        rows = [3 * k + 1 for k in range(ROWS_PER_CORE * c,
                                         ROWS_PER_CORE * (c + 1))]
        def shard(a):
            s = a[rows]                      # (4, 96, 96) = (i, j, z)
            flat = s.reshape(-1)             # flat = px*96 + z
            t = flat.reshape(NF, NP128).T    # [partition, free]
            return np.ascontiguousarray(t)
        in_maps.append({"xs": shard(X), "ys": shard(Y), "zs": shard(Z),
                        "bc": bcv, "sc": as_bf16(scv), "wr": as_bf16(wrv),
                        "sm": as_bf16(smv), "idm": as_bf16(np.eye(NT, dtype=np.float32))})

    res = run_bass_kernel_spmd(nc, in_maps, core_ids=list(range(N_CORES)))
    global LAST_EXEC_NS
    LAST_EXEC_NS = res.exec_time_ns

    parts = []
    for c in range(N_CORES):
        o = res.results[c]["out"]            # (64, 256) = [jj, i*64+v]
        parts.append(o.reshape(OUT_J, ROWS_PER_CORE, NV).transpose(1, 0, 2))
    return np.concatenate(parts, axis=0).astype(np.float32)  # (32, 64, 64)


# revision 11
# speedup vs baseline: 1.0668x; 1.0338x over previous
"""Trainium2 Bass kernel for the CubeSimulator problem (v2).

Reference: rotate (96,96,96) grids, per-voxel line-of-sight velocity u and
intensity I, Gaussian-KDE cube over 64 velocity bins, then trilinear
downsample (96,96,64) -> (32,64,64).

Exact structure reused from v1 (validated):
 - axis0 downsample (96->32) is a pure selection of rows 3k+1;
 - axis2 downsample (64->64) is the identity;
 - axis1 downsample (96->64) is a 2-tap stencil (0.75/0.25) matmul;
 - exp(L - (v-u)^2/sig^2) = exp(A) * exp(v*B + c_v) with
   A = L + ln(norm) - u^2/sig^2, B = 2u/sig^2, c_v = -v^2/sig^2.

New in v2 (tolerance-aware, rel err ~1.5e-3 vs 2e-2 budget):
 - Coarse-bin KDE: the cube is computed at NC=32 velocity centers and all
   64 reference bins are reconstructed with a ridge-regularized
   least-squares matrix R (a Gaussian with sigma=30 sampled at dv=19 is
   ~3x oversampled; aliasing ~2e-4). Halves the dominant per-bin work.
 - Wrap layout [128, 288]: per-core voxels flat=(px*96+z) are laid out
   partition=flat%128, free=flat//128, using all 128 lanes (elementwise
   engine cost scales with free size only). The z-sum for pixel p covers
   flat [96p, 96p+96), reduced on the (otherwise idle) TensorE with three
   accumulating matmuls per bin whose [128,4] selector stationaries are
   independent of the column triplet (128*3 = 96*4).
 - KDE tiles in bf16: DVE runs 2-byte tensor_tensor at 2x; exp args stay
   fp32 (ACT reads Bt fp32; scale/bias are per-bin immediates/APs).
 - Per-bin path: one ScalarE Exp (scale=vc, bias=c_v AP) + one V/P mult
   by P0 = exp(A). ScalarE is the bottleneck engine; all of prep's
   square/abs/ln/exp stay inside the natural_log_exp_and_others table.

Sharding: 32 needed i-rows split 4-per-core across 8 cores; only the
final (64, 4*64) tile is gathered per core.
"""

import math

import numpy as np

import concourse.bacc as bacc
import concourse.bass as bass
import concourse.mybir as mybir
import concourse.tile as tile
from concourse.bass_utils import run_bass_kernel_spmd

try:
    import ml_dtypes
    _BF16 = np.dtype(ml_dtypes.bfloat16)
except Exception:  # pragma: no cover
    _BF16 = None

G = 96            # up_gal grid size
NV = 64           # reference velocity bins
NC = 32           # coarse KDE bins (reconstructed to NV by matmul)
N_CORES = 8
OUT_I = 32        # selected i rows (axis-0 downsample = row selection)
ROWS_PER_CORE = OUT_I // N_CORES   # 4
PX = ROWS_PER_CORE * G             # 384 pixels per core
NZ = G                             # z depth
NFLAT = PX * NZ                    # 36864 voxels per core
NP128 = 128
NF = NFLAT // NP128                # 288 free columns
NT = NF // 3                       # 96 column triplets (4 pixels each)
OUT_J = 64
RIDGE_LAM = 1e-4

F32 = mybir.dt.float32
BF16 = mybir.dt.bfloat16
AF = mybir.ActivationFunctionType
OP = mybir.AluOpType

LAST_EXEC_NS = None

# tuning knobs
KDE_POOL_MULTS = 6   # of the NC bf16 P0-mults, how many go to GpSimd
PREP_FUSE_STT = False # use scalar_tensor_tensor fusion in prep where legal


def _build_program(ci, si, cr, sr, sig2, lnnorm, vel, safe_affine=None):
    vel = np.asarray(vel, np.float64).reshape(-1)
    vc = np.linspace(float(vel.min()), float(vel.max()), NC)
    usc = -200.0 * si
    if safe_affine is None:
        umax2 = (200.0 * si) ** 2
        safe_affine = not (umax2 / sig2 <= 80.0)
    # bins whose exp argument includes A directly (no P0 mult after).
    # The last bins are affine so the tail after the final exp is mult-free.
    affine_bins = set(range(NC)) if safe_affine else {NC - 2, NC - 1}

    nc = bacc.Bacc("TRN2")

    xs = nc.dram_tensor("xs", [NP128, NF], F32, kind="ExternalInput")
    ys = nc.dram_tensor("ys", [NP128, NF], F32, kind="ExternalInput")
    zs = nc.dram_tensor("zs", [NP128, NF], F32, kind="ExternalInput")
    bc = nc.dram_tensor("bc", [NP128, NC], F32, kind="ExternalInput")
    sc = nc.dram_tensor("sc", [NP128, 12], BF16, kind="ExternalInput")
    wr = nc.dram_tensor("wr", [NP128, 4 * NV], BF16, kind="ExternalInput")
    sm = nc.dram_tensor("sm", [NT, 16 * OUT_J], BF16, kind="ExternalInput")
    idm = nc.dram_tensor("idm", [NT, NT], BF16, kind="ExternalInput")
    out = nc.dram_tensor("out", [OUT_J, ROWS_PER_CORE * NV], F32,
                         kind="ExternalOutput")

    with tile.TileContext(nc) as tc:
        with (
            tc.tile_pool(name="io", bufs=1) as io,
            tc.tile_pool(name="prep", bufs=1) as prep,
            tc.tile_pool(name="kde", bufs=2) as kde,
            tc.tile_pool(name="psum", bufs=1, space="PSUM") as psum,
        ):
            # Preload the one activation table covering ln/exp/abs: avoids
            # two mid-kernel table swaps (~1.3us each) from the inserter's
            # minimal-set choice.
            from concourse.hw_specs import get_activation_tables
            tabs = get_activation_tables(nc.m.arch)
            want = {AF.Ln, AF.Exp, AF.Abs}
            for idx, (tname, funcs) in enumerate(tabs.items()):
                if want.issubset(funcs):
                    ld = mybir.InstLoadActFuncSet(
                        name=nc.scalar.bass.get_next_instruction_name(),
                        act_func_set_id=idx, ins=[], outs=[])
                    nc.scalar.add_instruction(ld)
                    break

            xt = io.tile([NP128, NF], F32, tag="xt")
            yt = io.tile([NP128, NF], F32, tag="yt")
            zt = io.tile([NP128, NF], F32, tag="zt")
            # input DMAs split in column halves so prep's first-half chain
            # starts ~0.7us earlier (HWDGE is serial; sem prop is ~0.9us)
            HF = NF // 2
            def half(ap, h):
                return ap[:, h * HF:(h + 1) * HF]
            for h in range(2):
                nc.sync.dma_start(out=half(zt, h), in_=half(zs, h))
                nc.sync.dma_start(out=half(xt, h), in_=half(xs, h))
                nc.sync.dma_start(out=half(yt, h), in_=half(ys, h))
            bct = io.tile([NP128, NC], F32, tag="bct")
            nc.sync.dma_start(out=bct[:], in_=bc[:])
            sct = io.tile([NP128, 12], BF16, tag="sct")
            nc.sync.dma_start(out=sct[:], in_=sc[:])
            idt = io.tile([NT, NT], BF16, tag="idt")
            nc.sync.dma_start(out=idt[:], in_=idm[:])
            wrt = io.tile([NP128, 4 * NV], BF16, tag="wrt")
            nc.sync.dma_start(out=wrt[:], in_=wr[:])
            smt = io.tile([NT, 16 * OUT_J], BF16, tag="smt")
            nc.sync.dma_start(out=smt[:], in_=sm[:])

            def vtile(name):
                return prep.tile([NP128, NF], F32, tag=name, name=name)

            # Prep, pipelined over two column halves to halve the serial
            # dependency chain's latency before the first KDE exp.
            # Only rx and rz legs are needed: r^2 = x^2+y^2+z^2 - rz^2
            # (rotation preserves |v|), so the ry leg is dropped.
            rx, rz = vtile("rx"), vtile("rz")
            sqx, sqy, sqz = vtile("sqx"), vtile("sqy"), vtile("sqz")
            s2, s3 = vtile("s2"), vtile("s3")
            xc, yc, t5, zb = vtile("xc"), vtile("yc"), vtile("t5"), vtile("zb")
            rzq, q, qs = vtile("rzq"), vtile("q"), vtile("qs")
            ya, xa = vtile("ya"), vtile("xa")
            lnq, r, er = vtile("lnq"), vtile("r"), vtile("er")
            ed, den, rec = vtile("ed"), vtile("den"), vtile("rec")
            num, t1, u0 = vtile("num"), vtile("t1"), vtile("u0")
            az, rterm, Lt = vtile("az"), vtile("rterm"), vtile("Lt")
            s1, ssq, At = vtile("s1"), vtile("ssq"), vtile("At")
            for h in range(2):
                V, P, S = nc.vector, nc.gpsimd, nc.scalar
                V.tensor_mul(half(sqz, h), half(zt, h), half(zt, h))
                V.tensor_scalar_mul(half(xc, h), half(xt, h), si * sr)
                V.tensor_mul(half(sqx, h), half(xt, h), half(xt, h))
                V.tensor_scalar_mul(half(yc, h), half(yt, h), si * cr)
                P.tensor_add(half(t5, h), half(xc, h), half(yc, h))
                P.tensor_scalar_mul(half(zb, h), half(zt, h), ci)
                V.tensor_mul(half(sqy, h), half(yt, h), half(yt, h))
                V.tensor_add(half(s2, h), half(sqx, h), half(sqy, h))
                V.tensor_add(half(s3, h), half(s2, h), half(sqz, h))
                V.tensor_add(half(rz, h), half(t5, h), half(zb, h))
                V.tensor_mul(half(rzq, h), half(rz, h), half(rz, h))
                V.tensor_sub(half(q, h), half(s3, h), half(rzq, h))
                V.tensor_scalar_max(half(qs, h), half(q, h), 1e-35)
                S.activation(half(lnq, h), half(qs, h), AF.Ln)
                S.activation(half(r, h), half(lnq, h), AF.Exp, scale=0.5)
                S.activation(half(er, h), half(r, h), AF.Exp)
                # rx for u0 (fits in DVE's idle slots)
                V.tensor_scalar_mul(half(ya, h), half(yt, h), -sr)
                V.tensor_scalar_mul(half(xa, h), half(xt, h), cr)
                V.tensor_add(half(rx, h), half(xa, h), half(ya, h))
                # u0 = rx*(e^r-1) / (r*(e^r+1))
                V.tensor_scalar_add(half(ed, h), half(er, h), 1.0)
                V.tensor_mul(half(den, h), half(ed, h), half(r, h))
                V.reciprocal(half(rec, h), half(den, h))
                P.tensor_scalar_add(half(num, h), half(er, h), -1.0)
                P.tensor_mul(half(t1, h), half(rx, h), half(num, h))
                V.tensor_mul(half(u0, h), half(t1, h), half(rec, h))
                # A = -r/3 - 2|rz| + lnnorm - (u*usc/sig)^2
                S.activation(half(az, h), half(rz, h), AF.Abs)
                P.tensor_scalar(half(rterm, h), half(r, h), -1.0 / 3.0,
                                lnnorm, OP.mult, OP.add)
                P.tensor_scalar_mul(half(az, h), half(az, h), -2.0)
                P.tensor_add(half(Lt, h), half(az, h), half(rterm, h))
                V.tensor_scalar_mul(half(s1, h), half(u0, h),
                                    usc / math.sqrt(sig2))
                V.tensor_mul(half(ssq, h), half(s1, h), half(s1, h))
                V.tensor_sub(half(At, h), half(Lt, h), half(ssq, h))

            P0t = vtile("P0t")
            P0b = prep.tile([NP128, NF], BF16, tag="P0b", name="P0b")

            # KDE over NC coarse bins; Op[t, 4b+m] accumulates the z-sums
            # (pixel p=4t+m of bin b) via three selector matmuls per bin.
            Op = psum.tile([NT, 4 * NC], F32)
            POOL_MULT_BINS = set(range(0, 2 * KDE_POOL_MULTS, 2))
            esc = usc * 2.0 / sig2
            pending = []

            def emit_tail(b, Ew):
                if b not in affine_bins:
                    e1 = Ew
                    Ew = kde.tile([NP128, NT, 3], BF16, tag="Ew", bufs=6,
                                  name=f"Ew{b}")
                    eng = nc.gpsimd if b in POOL_MULT_BINS else nc.vector
                    eng.tensor_mul(Ew[:], e1[:], P0b[:])
                for c in range(3):
                    nc.tensor.matmul(Op[:, 4 * b:4 * b + 4],
                                     Ew[:, :, c],
                                     sct[:, 4 * c:4 * c + 4],
                                     start=(c == 0), stop=(c == 2))

            # affine args for the designated bins (DVE slack, early emit is
            # fine: they only need u0 and At)
            aargs = {}
            for b in sorted(affine_bins):
                cvb = float(-vc[b] * vc[b] / sig2)
                tmp = kde.tile([NP128, NF], F32, tag=f"tmp{b}", bufs=1,
                               name=f"tmp{b}")
                nc.vector.tensor_scalar(tmp[:], u0[:], float(vc[b]) * esc,
                                        cvb, OP.mult, OP.add)
                arg = kde.tile([NP128, NF], F32, tag=f"arg{b}", bufs=1,
                               name=f"arg{b}")
                nc.vector.tensor_add(arg[:], tmp[:], At[:])
                aargs[b] = arg

            for b in range(NC):
                vv = float(vc[b])
                Ewd = kde.tile([NP128, NT, 3], BF16,
                               tag="Ew" if b in affine_bins else "e1",
                               bufs=6, name=f"e1{b}")
                if b in affine_bins:
                    nc.scalar.activation(Ewd[:], aargs[b][:], AF.Exp)
                else:
                    nc.scalar.activation(Ewd[:], u0[:], AF.Exp,
                                         scale=vv * esc,
                                         bias=bct[:, b:b + 1])
                pending.append((b, Ewd))
                if b == 1 and not safe_affine:
                    # P0 exp lands on ACT only now so the first KDE exps
                    # (gated only on u0) are not head-of-line blocked on At
                    nc.scalar.activation(P0t[:], At[:], AF.Exp)
                    nc.vector.tensor_copy(P0b[:], P0t[:])
                    for item in pending:
                        emit_tail(*item)
                    pending = []
                elif b > 1 or safe_affine:
                    for item in pending:
                        emit_tail(*item)
                    pending = []

                if b == NC // 2 - 1:
                    # first-half rearrange + W-matmul overlap the 2nd half
                    Ops0 = io.tile([NT, 2 * NC], BF16, tag="Ops0")
                    nc.vector.tensor_copy(Ops0[:], Op[:, 0:2 * NC])
                    Tp = psum.tile([4 * NC, NT], BF16)
                    nc.tensor.transpose(Tp[0:2 * NC, :], Ops0[:], idt[:])
                    Os = io.tile([NP128, NT], BF16, tag="Os")
                    nc.vector.tensor_copy(Os[0:2 * NC, :], Tp[0:2 * NC, :])
                    out1 = psum.tile([NT, 4 * NV], F32)
                    nc.tensor.matmul(out1[:], Os[0:2 * NC, :],
                                     wrt[0:2 * NC, :],
                                     start=True, stop=False)
            for item in pending:
                emit_tail(*item)

            # second-half rearrange, then cube2[px, v] closes in out1
            Ops1 = io.tile([NT, 2 * NC], BF16, tag="Ops1")
            nc.vector.tensor_copy(Ops1[:], Op[:, 2 * NC:4 * NC])
            nc.tensor.transpose(Tp[2 * NC:4 * NC, :], Ops1[:], idt[:])
            nc.vector.tensor_copy(Os[2 * NC:4 * NC, :], Tp[2 * NC:4 * NC, :])
            nc.tensor.matmul(out1[:], Os[2 * NC:4 * NC, :],
                             wrt[2 * NC:4 * NC, :],
                             start=False, stop=True)

            Os1 = io.tile([NT, 4 * NV], BF16, tag="Os1")
            nc.vector.tensor_copy(Os1[:], out1[:])

            # j-downsample: outf[jj, (i,v)] = sum_j wj[j,jj] cube2[96i+j, v]
            outf = psum.tile([OUT_J, ROWS_PER_CORE * NV], F32)
            for i in range(ROWS_PER_CORE):
                for m in range(4):
                    nc.tensor.matmul(outf[:, NV * i:NV * (i + 1)],
                                     smt[:, (i * 4 + m) * OUT_J:
                                         (i * 4 + m + 1) * OUT_J],
                                     Os1[:, NV * m:NV * (m + 1)],
                                     start=(m == 0), stop=(m == 3))
            outf_sb = io.tile([OUT_J, ROWS_PER_CORE * NV], F32, tag="outf_sb")
            nc.vector.tensor_copy(outf_sb[:], outf[:])
            nc.sync.dma_start(out=out[:], in_=outf_sb[:])

    return nc


def _recon_matrix(vel, sig2, si):
    """Ridge-regularized reconstruction R[NC, NV]: coarse Gaussian samples
    -> fine samples, fit over all reachable centers u."""
    vel = np.asarray(vel, np.float64).reshape(-1)
    vc = np.linspace(float(vel.min()), float(vel.max()), NC)
    umax = max(200.0 * abs(si), 1e-3)
    uu = np.linspace(-umax * 1.02, umax * 1.02, 4001)
    Ac = np.exp(-((vc[None, :] - uu[:, None]) ** 2) / sig2)
    Af = np.exp(-((vel[None, :] - uu[:, None]) ** 2) / sig2)
    R = np.linalg.solve(Ac.T @ Ac + RIDGE_LAM * np.eye(NC), Ac.T @ Af)
    return R.astype(np.float32)


def kernel(**inputs):
    inc = float(np.asarray(inputs["inclination"]).reshape(-1)[0])
    rot = float(np.asarray(inputs["sky_rot"]).reshape(-1)[0])
    lb = float(np.asarray(inputs["line_broadening"]).reshape(-1)[0])
    vel = np.asarray(inputs["velocity_grid"], np.float32).reshape(-1)
    X = np.asarray(inputs["Xgrid"], np.float32)
    Y = np.asarray(inputs["Ygrid"], np.float32)
    Z = np.asarray(inputs["Zgrid"], np.float32)

    ci, si = math.cos(inc), math.sin(inc)
    cr, sr = math.cos(rot), math.sin(rot)
    sig2 = float(np.float32(lb) * np.float32(lb))
    if not (sig2 > 0.0) or not math.isfinite(sig2):
        sig2 = 1e-30  # degenerate sigma: reference output is ~0/NaN anyway
    lnnorm = float(-0.5 * math.log(2.0 * math.pi * sig2))

    nc = _build_program(ci, si, cr, sr, sig2, lnnorm, vel)
    nc.finalize()

    vc = np.linspace(float(vel.min()), float(vel.max()), NC)
    bcv = np.ascontiguousarray(
        np.tile((-(vc.astype(np.float64) ** 2) / sig2).astype(np.float32),
                (NP128, 1)))

    # selector stationaries S_c
    scv = np.zeros((NP128, 12), np.float32)
    for c in range(3):
        for k in range(NP128):
            m = (128 * c + k) // 96
            if 0 <= m < 4 and 96 * m <= 128 * c + k < 96 * (m + 1):
                scv[k, 4 * c + m] = 1.0

    # reconstruction moving matrix W[(b,m), (m',v)] = delta R[b, v]
    R = _recon_matrix(vel, sig2, si)
    wrv = np.zeros((NP128, 4 * NV), np.float32)
    for b in range(NC):
        for m in range(4):
            wrv[4 * b + m, NV * m:NV * (m + 1)] = R[b]

    # j-downsample stencil and its zero-padded stationaries
    wj = np.zeros((G, OUT_J), np.float32)
    for m in range(OUT_J // 2):
        wj[3 * m, 2 * m] = 0.75
        wj[3 * m + 1, 2 * m] = 0.25
        wj[3 * m + 1, 2 * m + 1] = 0.25
        wj[3 * m + 2, 2 * m + 1] = 0.75
    smv = np.zeros((NT, 16 * OUT_J), np.float32)
    for i in range(4):
        for m in range(4):
            col = (i * 4 + m) * OUT_J
            for s in range(24):
                smv[24 * i + s, col:col + OUT_J] = wj[4 * s + m]

    as_bf16 = (lambda a: np.ascontiguousarray(a.astype(_BF16))) if _BF16 \
        else (lambda a: np.ascontiguousarray(a))

    in_maps = []
    for c in range(N_CORES):
        rows = [3 * k + 1 for k in range(ROWS_PER_CORE * c,
                                         ROWS_PER_CORE * (c + 1))]
        def shard(a):
            s = a[rows]                      # (4, 96, 96) = (i, j, z)
            flat = s.reshape(-1)             # flat = px*96 + z
            t = flat.reshape(NF, NP128).T    # [partition, free]
            return np.ascontiguousarray(t)
        in_maps.append({"xs": shard(X), "ys": shard(Y), "zs": shard(Z),
                        "bc": bcv, "sc": as_bf16(scv), "wr": as_bf16(wrv),
                        "sm": as_bf16(smv), "idm": as_bf16(np.eye(NT, dtype=np.float32))})

    res = run_bass_kernel_spmd(nc, in_maps, core_ids=list(range(N_CORES)))
    global LAST_EXEC_NS
    LAST_EXEC_NS = res.exec_time_ns

    parts = []
    for c in range(N_CORES):
        o = res.results[c]["out"]            # (64, 256) = [jj, i*64+v]
        parts.append(o.reshape(OUT_J, ROWS_PER_CORE, NV).transpose(1, 0, 2))
    return np.concatenate(parts, axis=0).astype(np.float32)  # (32, 64, 64)


# revision 12
# speedup vs baseline: 1.0821x; 1.0144x over previous
"""Trainium2 Bass kernel for the CubeSimulator problem (v2).

Reference: rotate (96,96,96) grids, per-voxel line-of-sight velocity u and
intensity I, Gaussian-KDE cube over 64 velocity bins, then trilinear
downsample (96,96,64) -> (32,64,64).

Exact structure reused from v1 (validated):
 - axis0 downsample (96->32) is a pure selection of rows 3k+1;
 - axis2 downsample (64->64) is the identity;
 - axis1 downsample (96->64) is a 2-tap stencil (0.75/0.25) matmul;
 - exp(L - (v-u)^2/sig^2) = exp(A) * exp(v*B + c_v) with
   A = L + ln(norm) - u^2/sig^2, B = 2u/sig^2, c_v = -v^2/sig^2.

New in v2 (tolerance-aware, rel err ~1.5e-3 vs 2e-2 budget):
 - Coarse-bin KDE: the cube is computed at NC=32 velocity centers and all
   64 reference bins are reconstructed with a ridge-regularized
   least-squares matrix R (a Gaussian with sigma=30 sampled at dv=19 is
   ~3x oversampled; aliasing ~2e-4). Halves the dominant per-bin work.
 - Wrap layout [128, 288]: per-core voxels flat=(px*96+z) are laid out
   partition=flat%128, free=flat//128, using all 128 lanes (elementwise
   engine cost scales with free size only). The z-sum for pixel p covers
   flat [96p, 96p+96), reduced on the (otherwise idle) TensorE with three
   accumulating matmuls per bin whose [128,4] selector stationaries are
   independent of the column triplet (128*3 = 96*4).
 - KDE tiles in bf16: DVE runs 2-byte tensor_tensor at 2x; exp args stay
   fp32 (ACT reads Bt fp32; scale/bias are per-bin immediates/APs).
 - Per-bin path: one ScalarE Exp (scale=vc, bias=c_v AP) + one V/P mult
   by P0 = exp(A). ScalarE is the bottleneck engine; all of prep's
   square/abs/ln/exp stay inside the natural_log_exp_and_others table.

Sharding: 32 needed i-rows split 4-per-core across 8 cores; only the
final (64, 4*64) tile is gathered per core.
"""

import math

import numpy as np

import concourse.bacc as bacc
import concourse.bass as bass
import concourse.mybir as mybir
import concourse.tile as tile
from concourse.bass_utils import run_bass_kernel_spmd

try:
    import ml_dtypes
    _BF16 = np.dtype(ml_dtypes.bfloat16)
except Exception:  # pragma: no cover
    _BF16 = None

G = 96            # up_gal grid size
NV = 64           # reference velocity bins
NC = 32           # coarse KDE bins (reconstructed to NV by matmul)
N_CORES = 8
OUT_I = 32        # selected i rows (axis-0 downsample = row selection)
ROWS_PER_CORE = OUT_I // N_CORES   # 4
PX = ROWS_PER_CORE * G             # 384 pixels per core
NZ = G                             # z depth
NFLAT = PX * NZ                    # 36864 voxels per core
NP128 = 128
NF = NFLAT // NP128                # 288 free columns
NT = NF // 3                       # 96 column triplets (4 pixels each)
OUT_J = 64
RIDGE_LAM = 1e-4

F32 = mybir.dt.float32
BF16 = mybir.dt.bfloat16
AF = mybir.ActivationFunctionType
OP = mybir.AluOpType

LAST_EXEC_NS = None

# tuning knobs
KDE_POOL_MULTS = 6   # of the NC bf16 P0-mults, how many go to GpSimd
PREP_FUSE_STT = False # use scalar_tensor_tensor fusion in prep where legal


def _build_program(ci, si, cr, sr, sig2, lnnorm, vel, safe_affine=None):
    vel = np.asarray(vel, np.float64).reshape(-1)
    vc = np.linspace(float(vel.min()), float(vel.max()), NC)
    usc = -200.0 * si
    if safe_affine is None:
        umax2 = (200.0 * si) ** 2
        safe_affine = not (umax2 / sig2 <= 80.0)
    # bins whose exp argument includes A directly (no P0 mult after).
    # The last bins are affine so the tail after the final exp is mult-free.
    affine_bins = set(range(NC)) if safe_affine else {NC - 2, NC - 1}

    nc = bacc.Bacc("TRN2")

    xs = nc.dram_tensor("xs", [NP128, NF], F32, kind="ExternalInput")
    ys = nc.dram_tensor("ys", [NP128, NF], F32, kind="ExternalInput")
    zs = nc.dram_tensor("zs", [NP128, NF], F32, kind="ExternalInput")
    bc = nc.dram_tensor("bc", [NP128, NC], F32, kind="ExternalInput")
    sc = nc.dram_tensor("sc", [NP128, 12], BF16, kind="ExternalInput")
    wr = nc.dram_tensor("wr", [NP128, 4 * NV], BF16, kind="ExternalInput")
    sm = nc.dram_tensor("sm", [NT, 16 * OUT_J], BF16, kind="ExternalInput")
    idm = nc.dram_tensor("idm", [NT, NT], BF16, kind="ExternalInput")
    out = nc.dram_tensor("out", [OUT_J, ROWS_PER_CORE * NV], F32,
                         kind="ExternalOutput")

    with tile.TileContext(nc) as tc:
        with (
            tc.tile_pool(name="io", bufs=1) as io,
            tc.tile_pool(name="prep", bufs=1) as prep,
            tc.tile_pool(name="kde", bufs=2) as kde,
            tc.tile_pool(name="psum", bufs=1, space="PSUM") as psum,
        ):
            # Preload the one activation table covering ln/exp/abs: avoids
            # two mid-kernel table swaps (~1.3us each) from the inserter's
            # minimal-set choice.
            from concourse.hw_specs import get_activation_tables
            tabs = get_activation_tables(nc.m.arch)
            want = {AF.Ln, AF.Exp, AF.Abs}
            for idx, (tname, funcs) in enumerate(tabs.items()):
                if want.issubset(funcs):
                    ld = mybir.InstLoadActFuncSet(
                        name=nc.scalar.bass.get_next_instruction_name(),
                        act_func_set_id=idx, ins=[], outs=[])
                    nc.scalar.add_instruction(ld)
                    break

            xt = io.tile([NP128, NF], F32, tag="xt")
            yt = io.tile([NP128, NF], F32, tag="yt")
            zt = io.tile([NP128, NF], F32, tag="zt")
            # input DMAs split in column halves so prep's first-half chain
            # starts ~0.7us earlier (HWDGE is serial; sem prop is ~0.9us)
            HF = NF // 2
            def half(ap, h):
                return ap[:, h * HF:(h + 1) * HF]
            for h in range(2):
                nc.sync.dma_start(out=half(zt, h), in_=half(zs, h))
                nc.sync.dma_start(out=half(xt, h), in_=half(xs, h))
                nc.sync.dma_start(out=half(yt, h), in_=half(ys, h))
            bct = io.tile([NP128, NC], F32, tag="bct")
            nc.sync.dma_start(out=bct[:], in_=bc[:])
            sct = io.tile([NP128, 12], BF16, tag="sct")
            nc.sync.dma_start(out=sct[:], in_=sc[:])
            idt = io.tile([NT, NT], BF16, tag="idt")
            nc.sync.dma_start(out=idt[:], in_=idm[:])
            wrt = io.tile([NP128, 4 * NV], BF16, tag="wrt")
            nc.sync.dma_start(out=wrt[:], in_=wr[:])
            smt = io.tile([NT, 16 * OUT_J], BF16, tag="smt")
            nc.sync.dma_start(out=smt[:], in_=sm[:])

            def vtile(name):
                return prep.tile([NP128, NF], F32, tag=name, name=name)

            # Prep, pipelined over two column halves to halve the serial
            # dependency chain's latency before the first KDE exp.
            # Only rx and rz legs are needed: r^2 = x^2+y^2+z^2 - rz^2
            # (rotation preserves |v|), so the ry leg is dropped.
            rx, rz = vtile("rx"), vtile("rz")
            sqx, sqy, sqz = vtile("sqx"), vtile("sqy"), vtile("sqz")
            s2, s3 = vtile("s2"), vtile("s3")
            xc, yc, t5, zb = vtile("xc"), vtile("yc"), vtile("t5"), vtile("zb")
            rzq, q, qs = vtile("rzq"), vtile("q"), vtile("qs")
            ya, xa = vtile("ya"), vtile("xa")
            lnq, r, er = vtile("lnq"), vtile("r"), vtile("er")
            ed, den, rec = vtile("ed"), vtile("den"), vtile("rec")
            num, t1, u0 = vtile("num"), vtile("t1"), vtile("u0")
            az, rterm, Lt = vtile("az"), vtile("rterm"), vtile("Lt")
            s1, ssq, At = vtile("s1"), vtile("ssq"), vtile("At")
            V, P, S = nc.vector, nc.gpsimd, nc.scalar
            for h in range(2):
                # geometry: q = x^2+y^2+z^2 - rz^2 and rx, per half
                V.tensor_mul(half(sqz, h), half(zt, h), half(zt, h))
                P.tensor_scalar_mul(half(xc, h), half(xt, h), si * sr)
                V.tensor_mul(half(sqx, h), half(xt, h), half(xt, h))
                P.tensor_scalar_mul(half(yc, h), half(yt, h), si * cr)
                P.tensor_add(half(t5, h), half(xc, h), half(yc, h))
                P.tensor_scalar_mul(half(zb, h), half(zt, h), ci)
                V.tensor_mul(half(sqy, h), half(yt, h), half(yt, h))
                V.tensor_add(half(s2, h), half(sqx, h), half(sqy, h))
                V.tensor_add(half(s3, h), half(s2, h), half(sqz, h))
                V.tensor_add(half(rz, h), half(t5, h), half(zb, h))
                V.tensor_mul(half(rzq, h), half(rz, h), half(rz, h))
                V.tensor_sub(half(q, h), half(s3, h), half(rzq, h))
                V.tensor_scalar_max(half(qs, h), half(q, h), 1e-35)
                V.tensor_scalar_mul(half(ya, h), half(yt, h), -sr)
                V.tensor_scalar_mul(half(xa, h), half(xt, h), cr)
                V.tensor_add(half(rx, h), half(xa, h), half(ya, h))
                # ACT ladder for this half
                S.activation(half(az, h), half(rz, h), AF.Abs)
                S.activation(half(lnq, h), half(qs, h), AF.Ln)
                S.activation(half(r, h), half(lnq, h), AF.Exp, scale=0.5)
                S.activation(half(er, h), half(r, h), AF.Exp)
                P.tensor_scalar_add(half(num, h), half(er, h), -1.0)
            for h in range(2):
                # u0 = rx*(e^r-1) / (r*(e^r+1)), per half
                V.tensor_scalar_add(half(ed, h), half(er, h), 1.0)
                V.tensor_mul(half(den, h), half(ed, h), half(r, h))
                V.reciprocal(half(rec, h), half(den, h))
                V.tensor_mul(half(t1, h), half(rx, h), half(num, h))
                V.tensor_mul(half(u0, h), half(t1, h), half(rec, h))
            for h in range(2):
                # A = -r/3 - 2|rz| + lnnorm - (u*usc/sig)^2
                P.tensor_scalar(half(rterm, h), half(r, h), -1.0 / 3.0,
                                lnnorm, OP.mult, OP.add)
                P.tensor_scalar_mul(half(az, h), half(az, h), -2.0)
                P.tensor_add(half(Lt, h), half(az, h), half(rterm, h))
                V.tensor_scalar_mul(half(s1, h), half(u0, h),
                                    usc / math.sqrt(sig2))
                V.tensor_mul(half(ssq, h), half(s1, h), half(s1, h))
                V.tensor_sub(half(At, h), half(Lt, h), half(ssq, h))

            P0t = vtile("P0t")
            P0b = prep.tile([NP128, NF], BF16, tag="P0b", name="P0b")

            # KDE over NC coarse bins; Op[t, 4b+m] accumulates the z-sums
            # (pixel p=4t+m of bin b) via three selector matmuls per bin.
            Op = psum.tile([NT, 4 * NC], F32)
            POOL_MULT_BINS = set(range(0, 2 * KDE_POOL_MULTS, 2))
            esc = usc * 2.0 / sig2
            pending = []

            def emit_tail(b, Ew):
                if b not in affine_bins:
                    e1 = Ew
                    Ew = kde.tile([NP128, NT, 3], BF16, tag="Ew", bufs=6,
                                  name=f"Ew{b}")
                    eng = nc.gpsimd if b in POOL_MULT_BINS else nc.vector
                    eng.tensor_mul(Ew[:], e1[:], P0b[:])
                for c in range(3):
                    nc.tensor.matmul(Op[:, 4 * b:4 * b + 4],
                                     Ew[:, :, c],
                                     sct[:, 4 * c:4 * c + 4],
                                     start=(c == 0), stop=(c == 2))

            # affine args for the designated bins (DVE slack, early emit is
            # fine: they only need u0 and At)
            aargs = {}
            for b in sorted(affine_bins):
                cvb = float(-vc[b] * vc[b] / sig2)
                tmp = kde.tile([NP128, NF], F32, tag=f"tmp{b}", bufs=1,
                               name=f"tmp{b}")
                nc.vector.tensor_scalar(tmp[:], u0[:], float(vc[b]) * esc,
                                        cvb, OP.mult, OP.add)
                arg = kde.tile([NP128, NF], F32, tag=f"arg{b}", bufs=1,
                               name=f"arg{b}")
                nc.vector.tensor_add(arg[:], tmp[:], At[:])
                aargs[b] = arg

            for b in range(NC):
                vv = float(vc[b])
                Ewd = kde.tile([NP128, NT, 3], BF16,
                               tag="Ew" if b in affine_bins else "e1",
                               bufs=6, name=f"e1{b}")
                if b in affine_bins:
                    nc.scalar.activation(Ewd[:], aargs[b][:], AF.Exp)
                else:
                    nc.scalar.activation(Ewd[:], u0[:], AF.Exp,
                                         scale=vv * esc,
                                         bias=bct[:, b:b + 1])
                pending.append((b, Ewd))
                if b == 1 and not safe_affine:
                    # P0 exp lands on ACT only now so the first KDE exps
                    # (gated only on u0) are not head-of-line blocked on At
                    nc.scalar.activation(P0t[:], At[:], AF.Exp)
                    nc.vector.tensor_copy(P0b[:], P0t[:])
                    for item in pending:
                        emit_tail(*item)
                    pending = []
                elif b > 1 or safe_affine:
                    for item in pending:
                        emit_tail(*item)
                    pending = []

                if b == NC // 2 - 1:
                    # first-half rearrange + W-matmul overlap the 2nd half
                    Ops0 = io.tile([NT, 2 * NC], BF16, tag="Ops0")
                    nc.vector.tensor_copy(Ops0[:], Op[:, 0:2 * NC])
                    Tp = psum.tile([4 * NC, NT], BF16)
                    nc.tensor.transpose(Tp[0:2 * NC, :], Ops0[:], idt[:])
                    Os = io.tile([NP128, NT], BF16, tag="Os")
                    nc.vector.tensor_copy(Os[0:2 * NC, :], Tp[0:2 * NC, :])
                    out1 = psum.tile([NT, 4 * NV], F32)
                    nc.tensor.matmul(out1[:], Os[0:2 * NC, :],
                                     wrt[0:2 * NC, :],
                                     start=True, stop=False)
            for item in pending:
                emit_tail(*item)

            # second-half rearrange, then cube2[px, v] closes in out1
            Ops1 = io.tile([NT, 2 * NC], BF16, tag="Ops1")
            nc.vector.tensor_copy(Ops1[:], Op[:, 2 * NC:4 * NC])
            nc.tensor.transpose(Tp[2 * NC:4 * NC, :], Ops1[:], idt[:])
            nc.vector.tensor_copy(Os[2 * NC:4 * NC, :], Tp[2 * NC:4 * NC, :])
            nc.tensor.matmul(out1[:], Os[2 * NC:4 * NC, :],
                             wrt[2 * NC:4 * NC, :],
                             start=False, stop=True)

            Os1 = io.tile([NT, 4 * NV], BF16, tag="Os1")
            nc.vector.tensor_copy(Os1[:], out1[:])

            # j-downsample: outf[jj, (i,v)] = sum_j wj[j,jj] cube2[96i+j, v]
            outf = psum.tile([OUT_J, ROWS_PER_CORE * NV], F32)
            for i in range(ROWS_PER_CORE):
                for m in range(4):
                    nc.tensor.matmul(outf[:, NV * i:NV * (i + 1)],
                                     smt[:, (i * 4 + m) * OUT_J:
                                         (i * 4 + m + 1) * OUT_J],
                                     Os1[:, NV * m:NV * (m + 1)],
                                     start=(m == 0), stop=(m == 3))
            outf_sb = io.tile([OUT_J, ROWS_PER_CORE * NV], F32, tag="outf_sb")
            nc.vector.tensor_copy(outf_sb[:], outf[:])
            nc.sync.dma_start(out=out[:], in_=outf_sb[:])

    return nc


def _recon_matrix(vel, sig2, si):
    """Ridge-regularized reconstruction R[NC, NV]: coarse Gaussian samples
    -> fine samples, fit over all reachable centers u."""
    vel = np.asarray(vel, np.float64).reshape(-1)
    vc = np.linspace(float(vel.min()), float(vel.max()), NC)
    umax = max(200.0 * abs(si), 1e-3)
    uu = np.linspace(-umax * 1.02, umax * 1.02, 4001)
    Ac = np.exp(-((vc[None, :] - uu[:, None]) ** 2) / sig2)
    Af = np.exp(-((vel[None, :] - uu[:, None]) ** 2) / sig2)
    R = np.linalg.solve(Ac.T @ Ac + RIDGE_LAM * np.eye(NC), Ac.T @ Af)
    return R.astype(np.float32)


def kernel(**inputs):
    inc = float(np.asarray(inputs["inclination"]).reshape(-1)[0])
    rot = float(np.asarray(inputs["sky_rot"]).reshape(-1)[0])
    lb = float(np.asarray(inputs["line_broadening"]).reshape(-1)[0])
    vel = np.asarray(inputs["velocity_grid"], np.float32).reshape(-1)
    X = np.asarray(inputs["Xgrid"], np.float32)
    Y = np.asarray(inputs["Ygrid"], np.float32)
    Z = np.asarray(inputs["Zgrid"], np.float32)

    ci, si = math.cos(inc), math.sin(inc)
    cr, sr = math.cos(rot), math.sin(rot)
    sig2 = float(np.float32(lb) * np.float32(lb))
    if not (sig2 > 0.0) or not math.isfinite(sig2):
        sig2 = 1e-30  # degenerate sigma: reference output is ~0/NaN anyway
    lnnorm = float(-0.5 * math.log(2.0 * math.pi * sig2))

    nc = _build_program(ci, si, cr, sr, sig2, lnnorm, vel)
    nc.finalize()

    vc = np.linspace(float(vel.min()), float(vel.max()), NC)
    bcv = np.ascontiguousarray(
        np.tile((-(vc.astype(np.float64) ** 2) / sig2).astype(np.float32),
                (NP128, 1)))

    # selector stationaries S_c
    scv = np.zeros((NP128, 12), np.float32)
    for c in range(3):
        for k in range(NP128):
            m = (128 * c + k) // 96
            if 0 <= m < 4 and 96 * m <= 128 * c + k < 96 * (m + 1):
                scv[k, 4 * c + m] = 1.0

    # reconstruction moving matrix W[(b,m), (m',v)] = delta R[b, v]
    R = _recon_matrix(vel, sig2, si)
    wrv = np.zeros((NP128, 4 * NV), np.float32)
    for b in range(NC):
        for m in range(4):
            wrv[4 * b + m, NV * m:NV * (m + 1)] = R[b]

    # j-downsample stencil and its zero-padded stationaries
    wj = np.zeros((G, OUT_J), np.float32)
    for m in range(OUT_J // 2):
        wj[3 * m, 2 * m] = 0.75
        wj[3 * m + 1, 2 * m] = 0.25
        wj[3 * m + 1, 2 * m + 1] = 0.25
        wj[3 * m + 2, 2 * m + 1] = 0.75
    smv = np.zeros((NT, 16 * OUT_J), np.float32)
    for i in range(4):
        for m in range(4):
            col = (i * 4 + m) * OUT_J
            for s in range(24):
                smv[24 * i + s, col:col + OUT_J] = wj[4 * s + m]

    as_bf16 = (lambda a: np.ascontiguousarray(a.astype(_BF16))) if _BF16 \
        else (lambda a: np.ascontiguousarray(a))

    in_maps = []
    for c in range(N_CORES):
        rows = [3 * k + 1 for k in range(ROWS_PER_CORE * c,
                                         ROWS_PER_CORE * (c + 1))]
        def shard(a):
            s = a[rows]                      # (4, 96, 96) = (i, j, z)
            flat = s.reshape(-1)             # flat = px*96 + z
            t = flat.reshape(NF, NP128).T    # [partition, free]
            return np.ascontiguousarray(t)
        in_maps.append({"xs": shard(X), "ys": shard(Y), "zs": shard(Z),
                        "bc": bcv, "sc": as_bf16(scv), "wr": as_bf16(wrv),
                        "sm": as_bf16(smv), "idm": as_bf16(np.eye(NT, dtype=np.float32))})

    res = run_bass_kernel_spmd(nc, in_maps, core_ids=list(range(N_CORES)))
    global LAST_EXEC_NS
    LAST_EXEC_NS = res.exec_time_ns

    parts = []
    for c in range(N_CORES):
        o = res.results[c]["out"]            # (64, 256) = [jj, i*64+v]
        parts.append(o.reshape(OUT_J, ROWS_PER_CORE, NV).transpose(1, 0, 2))
    return np.concatenate(parts, axis=0).astype(np.float32)  # (32, 64, 64)
